# revision 48
# baseline (speedup 1.0000x reference)
"""Bass/Trainium2 kernel for nn_EnhancedBilinearInteraction.

Reference computation:
    xp = W2 @ (W1 @ x[b,l,:] + b1) + b2      (per token, -> [B, 512, L])
    yp = same for y
    out[b,h] = sum_l tanh(xp[b,h,l]) * tanh(yp[b,h,l])

Host-side algebraic rewrite: the two 1x1 convs are consecutive linear maps,
so W_eff = W2 @ W1 ([512, 256]) and b_eff = W2 @ b1 + b2 ([512]) give the
same projection with 3x fewer FLOPs. Inputs are pre-transposed to
channel-major and cast to fp16 on the host (PE runs fp16 at full rate).

Sharding: pure data parallel - batch dim (32) split across 8 cores.
"""

import numpy as np

B, L, C, H = 32, 2048, 256, 512
NCORES = 8
BPC = B // NCORES          # 4 batches per core
TOK = BPC * L              # 8192 tokens per core
CHUNK = L                  # one batch per chunk -> reduce maps 1:1 to out col
KT = C // 128              # 2 contraction tiles
MT = H // 128              # 4 output-row tiles
NSUB = CHUNK // 512        # 4 matmul sub-chunks per chunk
NCH = TOK // CHUNK         # 4 chunks per core

_CACHE = {}


def _build():
    import concourse.tile as tile
    from concourse import bacc, mybir

    nc = bacc.Bacc(
        "TRN2", target_bir_lowering=False, debug=False, num_devices=NCORES
    )
    xT = nc.dram_tensor("xT", [C, TOK], mybir.dt.float16, kind="ExternalInput").ap()
    yT = nc.dram_tensor("yT", [C, TOK], mybir.dt.float16, kind="ExternalInput").ap()
    wT = nc.dram_tensor("wT", [C, H], mybir.dt.float16, kind="ExternalInput").ap()
    bE = nc.dram_tensor("bE", [H], mybir.dt.float32, kind="ExternalInput").ap()
    out = nc.dram_tensor("out", [BPC, H], mybir.dt.float32, kind="ExternalOutput").ap()

    Tanh = mybir.ActivationFunctionType.Tanh

    with tile.TileContext(nc) as tc:
        with (
            tc.tile_pool(name="singles", bufs=1) as singles,
            tc.tile_pool(name="acts", bufs=6) as apool,
            tc.tile_pool(name="scratch", bufs=2) as spool,
            tc.tile_pool(name="psum", bufs=2, space="PSUM") as pspool,
        ):
            wt = singles.tile([128, KT, H], mybir.dt.float16)
            bsb = singles.tile([128, MT], mybir.dt.float32)
            out_sb = singles.tile([128, MT, BPC], mybir.dt.float32)
            out_fin = singles.tile([128, MT, BPC], mybir.dt.float32)

            xt = singles.tile([128, KT, TOK], mybir.dt.float16)
            yt = singles.tile([128, KT, TOK], mybir.dt.float16)
            # Per-(k, chunk) input DMAs. The DMA engines are the serial
            # resource early on, so order: x chunk 0, y chunk 0, then the
            # rest - the first matmuls only wait on a 512KB transfer.
            def load(dst, src, j, k, lo=0, hi=CHUNK, eng=None):
                sl = slice(j * CHUNK + lo, j * CHUNK + hi)
                (eng or nc.sync).dma_start(
                    out=dst[:, k, sl], in_=src[k * 128 : (k + 1) * 128, sl]
                )

            nc.sync.dma_start(out=wt, in_=wT.rearrange("(t p) m -> p t m", p=128))
            for piece in range(2):
                for k in range(KT):
                    load(xt, xT, 0, k, piece * 1024, (piece + 1) * 1024)
            nc.sync.dma_start(out=bsb, in_=bE.rearrange("(m p) -> p m", p=128))
            for k in range(KT):
                load(yt, yT, 0, k)
            for j in range(1, NCH):
                for k in range(KT):
                    load(xt, xT, j, k)
                for k in range(KT):
                    load(yt, yT, j, k)

            # PE warmup: junk matmuls on zeroed data ramp the tensor engine
            # toward full clock while the first input DMA is in flight.
            junk = singles.tile([128, 128], mybir.dt.float16)
            nc.vector.memset(junk, 0.0)
            psjunk = pspool.tile([128, 96], mybir.dt.float32, tag="ps")
            for _ in range(42):
                nc.tensor.matmul(
                    psjunk, junk, junk[:, :96], start=True, stop=True
                )
            # Dummy activation to pull the ~1.3us tanh table load off the
            # critical path (it otherwise runs right before the first tanh).
            junk_act = singles.tile([128, 1], mybir.dt.float16)
            nc.scalar.activation(junk_act, junk[:, :1], Tanh, bias=0.0)

            def project(src, j, m):
                """psum[:, l] = sum_k W_eff[m-tile].T @ src[k-tile, chunk j]"""
                ps = pspool.tile([128, CHUNK], mybir.dt.float32, tag="ps")
                for n in range(NSUB):
                    for k in range(KT):
                        nc.tensor.matmul(
                            ps[:, n * 512 : (n + 1) * 512],
                            wt[:, k, m * 128 : (m + 1) * 128],
                            src[:, k, j * CHUNK + n * 512 : j * CHUNK + (n + 1) * 512],
                            start=(k == 0),
                            stop=(k == KT - 1),
                        )
                return ps

            from concourse.dve_ops import TENSOR_TENSOR_REDUCE

            def tail_reduce(xa, ya, m, j):
                # Fused multiply+reduce in one DVE pass via the custom-ucode
                # TENSOR_TENSOR_REDUCE op (the raw ISA tensor_tensor_reduce
                # opcode fails on this runtime's hardware path).
                prod = spool.tile([128, CHUNK], mybir.dt.float16, tag="prod")
                nc.vector._custom_dve(
                    TENSOR_TENSOR_REDUCE,
                    out=prod,
                    in0=xa,
                    in1=ya,
                    s0=0.0,
                    s1=1.0,
                    accum_out=out_sb[:, m, j : j + 1],
                )

            def x_proj(j, m):
                xa = apool.tile([128, CHUNK], mybir.dt.float16, tag="xa")
                ps_x = project(xt, j, m)
                nc.scalar.activation(xa, ps_x, Tanh, bias=bsb[:, m : m + 1])
                return xa

            def y_proj(j, m):
                ps_y = project(yt, j, m)
                ya = apool.tile([128, CHUNK], mybir.dt.float16, tag="ya")
                nc.scalar.activation(ya, ps_y, Tanh, bias=bsb[:, m : m + 1])
                return ya

            # Chunk 0: all x-projections first so ACT saturates while y's
            # DMA is still in flight. Later chunks (data resident): x/y
            # interleaved per m so the DVE chain never lags ACT at the end.
            xas = [x_proj(0, m) for m in range(MT)]
            for m in range(MT):
                tail_reduce(xas[m], y_proj(0, m), m, 0)
            for j in range(1, NCH):
                for m in range(MT):
                    xa = x_proj(j, m)
                    if j == NCH - 1 and m == MT - 1:
                        # Final iteration: halve the y-activation and reduce
                        # so the end-of-kernel serial chain is half as long.
                        ps_y = project(yt, j, m)
                        ya = apool.tile([128, CHUNK], mybir.dt.float16, tag="ya")
                        h = CHUNK // 2
                        acc = out_sb[:, m, j : j + 1]
                        bias = bsb[:, m : m + 1]
                        prod = spool.tile(
                            [128, CHUNK], mybir.dt.float16, tag="prod"
                        )
                        nc.scalar.activation(
                            ya[:, :h], ps_y[:, :h], Tanh, bias=bias
                        )
                        nc.vector._custom_dve(
                            TENSOR_TENSOR_REDUCE,
                            out=prod[:, :h],
                            in0=xa[:, :h],
                            in1=ya[:, :h],
                            s0=0.0,
                            s1=1.0,
                            accum_out=acc,
                        )
                        nc.scalar.activation(
                            ya[:, h:], ps_y[:, h:], Tanh, bias=bias
                        )
                        nc.vector._custom_dve(
                            TENSOR_TENSOR_REDUCE,
                            out=prod[:, h:],
                            in0=xa[:, h:],
                            in1=ya[:, h:],
                            s0=acc,
                            s1=1.0,
                            accum_out=acc,
                        )
                    else:
                        tail_reduce(xa, y_proj(j, m), m, j)
                    if j == NCH - 1:
                        # out_sb's only writers are accum_out (second output
                        # slot) of the custom DVE ops. Funnel each m through a
                        # DVE copy: the DVE is serial, so the copy runs after
                        # every reduce for this m, and the output DMA then
                        # depends on a standard first-output write. Closes an
                        # intermittent output race observed on HW.
                        nc.vector.tensor_copy(
                            out_fin[:, m, :], out_sb[:, m, :]
                        )

            for m in range(MT):
                nc.sync.dma_start(
                    out=out[:, m * 128 : (m + 1) * 128].rearrange("b p -> p b"),
                    in_=out_fin[:, m, :],
                )
    nc.compile()
    return nc


def _prep_inputs(x, y, W1, b1, W2, b2):
    x, y, W1, b1, W2, b2 = (
        np.asarray(t, dtype=np.float32) for t in (x, y, W1, b1, W2, b2)
    )
    W_eff = W2.astype(np.float64) @ W1.astype(np.float64)        # [H, C]
    b_eff = (W2.astype(np.float64) @ b1.astype(np.float64) + b2).astype(np.float32)
    wT_np = np.ascontiguousarray(W_eff.T).astype(np.float16)      # [C, H]

    in_maps = []
    for i in range(NCORES):
        xs = x[i * BPC : (i + 1) * BPC].reshape(TOK, C)
        ys = y[i * BPC : (i + 1) * BPC].reshape(TOK, C)
        in_maps.append(
            {
                "xT": np.ascontiguousarray(xs.T).astype(np.float16),
                "yT": np.ascontiguousarray(ys.T).astype(np.float16),
                "wT": wT_np,
                "bE": b_eff,
            }
        )
    return in_maps


def _run(inputs, trace=False):
    from concourse.bass_utils import run_bass_kernel_spmd

    if "nc" not in _CACHE:
        _CACHE["nc"] = _build()
    nc = _CACHE["nc"]
    in_maps = _prep_inputs(**inputs)
    # Retry once: transient device wedges (NRT_EXEC_UNIT_UNRECOVERABLE)
    # have been observed under heavy back-to-back use and usually clear
    # on re-execution.
    last_exc = None
    for _ in range(2):
        try:
            res = run_bass_kernel_spmd(
                nc, in_maps, core_ids=list(range(NCORES)), trace=trace
            )
            break
        except Exception as e:  # noqa: BLE001
            last_exc = e
    else:
        raise last_exc
    full = np.concatenate([r["out"] for r in res.results], axis=0)  # [B, H]
    return full, res


def kernel(x, y, W1, b1, W2, b2):
    full, _ = _run(dict(x=x, y=y, W1=W1, b1=b1, W2=W2, b2=b2))
    return full
